# revision 1
# baseline (speedup 1.0000x reference)
"""CBAM block (channel + spatial attention) Trainium2 Bass kernel.

Problem: x [32, 56, 56, 256] f32; data-parallel over batch across 8 NeuronCores
(4 images per core).  Everything is hardcoded for these shapes.

Per-core dataflow (B=4 images, each [3136(hw), 256(c)] f32, kept resident in SBUF):
  layout: X[p, b, t, c] with p in [0,128) partitions, t in [0,25) blocks,
          flat row n = 128*t + p (block 24 is half: rows 3072..3135 -> p<64).

  Stage A (channel attention stats, per image):
    - max over hw: DVE tensor_reduce over t (blocks 0..23, t innermost) ->
      [128, 256], combine half block, then GPSIMD partition_all_reduce(max).
    - sum over hw: PE ones-matmul (one-hot lhsT so the sum lands on psum row 0),
      accumulated over the 25 blocks.
  MLP (per image, tiny): stats [2, 256] -> PE transpose -> [256, 2] ->
    W1/relu/W2 matmuls -> sigmoid(colA + colM + 2*b2) -> ca^T [256] ->
    PE transpose + ones-broadcast matmul -> bca [128, 256].
  Stage B (per image, per block):
    - DVE tensor_tensor_reduce: xr = x * bca (in place) + accum max over c.
    - ACT copy with accum_out: sum over c (mean map after 1/256 scale).
  Spatial conv 7x7 (per image): maps [128, 25] -> DRAM -> [56(w), 56(h)] tiles;
    conv = 14 accumulated PE matmuls with per-(ch,dh) Toeplitz lhsT built at
    runtime from conv_w via 98 tiny strided DMA band-writes into DRAM.
    sigmoid on ACT, then rearrange back to flat [128, 25].
  Apply: out = xr * sa (per-partition scalar per block) split DVE/ACT; DMA out.
"""

import os

import numpy as np

import concourse.bass as bass
import concourse.bacc as bacc
import concourse.bass_isa as bass_isa
import concourse.tile as tile
from concourse import mybir
from concourse.bass_utils import run_bass_kernel_spmd

F32 = mybir.dt.float32
AX = mybir.AxisListType
OP = mybir.AluOpType
ACT = mybir.ActivationFunctionType

P = 128          # partitions per block
NB = 25          # blocks per image (24 full + 1 half)
NBF = 24         # full blocks
HALF = 64        # valid rows in block 24
C = 256          # channels
HW = 3136        # 56*56
NIMG = 4         # images per core
NCORES = 8

_CACHE: dict = {}

# CBAM_STAGE: 5 = full kernel (default); lower values truncate the program for
# hardware bisection: 1 = DMA passthrough, 2 = +phase A, 3 = +phase B stats,
# 4 = +conv/apply but with zeroed Toeplitz bands (no band DMAs).
def _stage() -> int:
    return int(os.environ.get("CBAM_STAGE", "5"))


def _bsub() -> int:
    # sub-bisection inside phase B: 1=TTR/sumc only, 2=+map rearrange,
    # 3=+conv+sa rearrange, 4=full (apply + out)
    return int(os.environ.get("CBAM_B_SUB", "4"))


def _pp(t: int) -> int:
    return P if t < NBF else HALF


def _build_nc() -> bass.Bass:
    nc = bacc.Bacc()

    x_d = nc.dram_tensor("x", [NIMG, 56, 56, C], F32, kind="ExternalInput")
    w1_d = nc.dram_tensor("w1", [C, 16], F32, kind="ExternalInput")
    b1_d = nc.dram_tensor("b1", [16], F32, kind="ExternalInput")
    w2_d = nc.dram_tensor("w2", [16, C], F32, kind="ExternalInput")
    b2_d = nc.dram_tensor("b2", [C], F32, kind="ExternalInput")
    cw_d = nc.dram_tensor("conv_w", [7, 7, 2, 1], F32, kind="ExternalInput")
    out_d = nc.dram_tensor("out", [NIMG, 56, 56, C], F32, kind="ExternalOutput")

    ident_d = nc.inline_tensor(np.eye(128, dtype=np.float32), name="ident128")

    x_hwc = x_d[:].rearrange("b h w c -> b (h w) c")
    out_hwc = out_d[:].rearrange("b h w c -> b (h w) c")

    with tile.TileContext(nc) as tc:
        import contextlib

        with contextlib.ExitStack() as ctx:
            cpool = ctx.enter_context(tc.tile_pool(name="cpool", bufs=1))
            xpool = ctx.enter_context(tc.tile_pool(name="xpool", bufs=1))
            work = ctx.enter_context(tc.tile_pool(name="work", bufs=3))
            small = ctx.enter_context(tc.tile_pool(name="small", bufs=3))
            mappool = ctx.enter_context(tc.tile_pool(name="mappool", bufs=2))
            psA = ctx.enter_context(tc.tile_pool(name="psA", bufs=2, space="PSUM"))
            psB = ctx.enter_context(tc.tile_pool(name="psB", bufs=2, space="PSUM"))
            psM = ctx.enter_context(tc.tile_pool(name="psM", bufs=2, space="PSUM"))
            dpool = ctx.enter_context(tc.tile_pool(name="dpool", bufs=2, space="DRAM"))
            dpool1 = ctx.enter_context(tc.tile_pool(name="dpool1", bufs=1, space="DRAM"))

            # ---------------- constants & weights ----------------
            ident = cpool.tile([128, 128], F32)
            nc.sync.dma_start(out=ident, in_=ident_d[:])

            w1_sb = cpool.tile([128, 2, 16], F32)
            nc.sync.dma_start(out=w1_sb, in_=w1_d[:].rearrange("(j p) m -> p j m", p=128))
            w2_sb = cpool.tile([16, 2, 128], F32)
            nc.sync.dma_start(out=w2_sb, in_=w2_d[:].rearrange("k (j m) -> k j m", j=2))
            b1_sb = cpool.tile([16, 1], F32)
            nc.sync.dma_start(out=b1_sb, in_=b1_d[:].rearrange("(p o) -> p o", o=1))
            b2_sb = cpool.tile([128, 2], F32)
            nc.sync.dma_start(out=b2_sb, in_=b2_d[:].rearrange("(j p) -> p j", p=128))
            b2x2 = cpool.tile([128, 2], F32)
            nc.scalar.activation(out=b2x2, in_=b2_sb, func=ACT.Copy, scale=2.0)

            oh2 = cpool.tile([128, 2], F32)
            nc.vector.memset(oh2[:, 0:1], 1.0)
            nc.vector.memset(oh2[:, 1:2], 0.0)
            ones_r = cpool.tile([1, 128], F32)
            nc.vector.memset(ones_r, 1.0)

            # ---------------- Toeplitz conv matrices in DRAM ----------------
            # T[ch, dh][w_in, w_out] = conv_w[dh+3, dw+3, ch] where dw = w_in - w_out
            t_dram = dpool1.tile([2 * 7 * 56 * 56], F32)
            zt = cpool.tile([128, 343], F32)
            nc.vector.memset(zt, 0.0)
            nc.sync.dma_start(
                out=t_dram.rearrange("(p t) -> p t", p=128), in_=zt
            )
            for ch in (range(2) if _stage() >= 5 else ()):
                for dhi in range(7):
                    for dwi in range(7):
                        dw = dwi - 3
                        cnt = 56 - abs(dw)
                        w0 = max(0, -dw)
                        off = (ch * 7 + dhi) * 3136 + 57 * w0 + 56 * dw
                        dst = bass.AP(
                            tensor=t_dram.tensor,
                            offset=t_dram.offset + off,
                            ap=[[57, cnt]],
                        )
                        src = bass.AP(
                            tensor=cw_d,
                            offset=dhi * 14 + dwi * 2 + ch,
                            ap=[[0, cnt]],
                        )
                        nc.sync.dma_start(out=dst, in_=src)
            t_sb = cpool.tile([56, 14, 56], F32)
            nc.sync.dma_start(
                out=t_sb,
                in_=t_dram.rearrange("(m w v) -> w m v", m=14, w=56),
            )

            # ---------------- big SBUF state ----------------
            X = xpool.tile([P, NIMG, NB, C], F32)
            bca = cpool.tile([P, NIMG, C], F32)
            scr = cpool.tile([P, C], F32)  # ACT dummy-copy target
            scr2 = cpool.tile([P, C], F32)  # DVE dummy target for bisection

            # PE warm-up matmuls touching every constant lhsT source so that
            # later matmuls don't accumulate one sync-wait per constant tensor
            # (the LDW struct has very few wait slots).
            pwu = psM.tile([128, 4], F32, tag="mlp")
            nc.tensor.matmul(pwu[0:2, 0:2], lhsT=oh2, rhs=oh2, start=True, stop=True)
            nc.tensor.matmul(
                pwu[0:4, 0:4],
                lhsT=ident[:, 0:4],
                rhs=ident[:, 0:4],
                start=True,
                stop=True,
            )
            nc.tensor.matmul(
                pwu[0:128, 0:1],
                lhsT=ones_r.rearrange("p m -> p m"),
                rhs=ones_r[:, 0:1],
                start=True,
                stop=True,
            )
            nc.tensor.matmul(
                pwu[0:4, 0:4],
                lhsT=t_sb[:, 0, 0:4],
                rhs=t_sb[:, 0, 0:4],
                start=True,
                stop=True,
            )
            nc.tensor.matmul(
                pwu[0:4, 0:4],
                lhsT=w1_sb[:, 0, 0:4],
                rhs=w1_sb[:, 0, 0:4],
                start=True,
                stop=True,
            )
            nc.tensor.matmul(
                pwu[0:4, 0:4],
                lhsT=w2_sb[:, 0, 0:4],
                rhs=w2_sb[:, 0, 0:4],
                start=True,
                stop=True,
            )

            # ---------------- DMA in (all images up front) ----------------
            for b in range(NIMG):
                nc.sync.dma_start(
                    out=X[:, b, 0:NBF, :],
                    in_=x_hwc[b, 0 : NBF * P, :].rearrange("(t p) c -> p t c", p=128),
                )
                nc.sync.dma_start(
                    out=X[0:HALF, b, NBF, :], in_=x_hwc[b, NBF * P : HW, :]
                )

            # ---------------- phase A + MLP per image ----------------
            for b in (range(NIMG) if _stage() >= 2 else ()):
                # ---- max over hw ----
                acc = work.tile([P, C], F32, tag="acc")
                nc.vector.tensor_reduce(
                    out=acc,
                    in_=X[:, b, 0:NBF, :].rearrange("p t c -> p c t"),
                    axis=AX.X,
                    op=OP.max,
                )
                nc.vector.tensor_max(
                    out=acc[0:HALF], in0=acc[0:HALF], in1=X[0:HALF, b, NBF, :]
                )
                allred = work.tile([P, C], F32, tag="allred")
                nc.gpsimd.partition_all_reduce(allred, acc, 128, bass_isa.ReduceOp.max)

                # ---- sum over hw on PE (lands on psum partition row 0) ----
                ps = psA.tile([2, C], F32, tag="ps_sum")
                for t in range(NBF):
                    nc.tensor.matmul(
                        ps, lhsT=oh2, rhs=X[:, b, t, :], start=(t == 0), stop=False
                    )
                nc.tensor.matmul(
                    ps,
                    lhsT=oh2[0:HALF],
                    rhs=X[0:HALF, b, NBF, :],
                    start=False,
                    stop=True,
                )

                # ---- stats [2, 256]: row0 = avg, row1 = max ----
                stats = small.tile([2, C], F32, tag="stats")
                # allred holds the hw-max on every partition; fill both rows
                # with it, then overwrite row 0 with the avg (issue order keeps
                # the writes correctly sequenced).
                nc.scalar.activation(out=stats, in_=allred[0:2, :], func=ACT.Copy)
                nc.scalar.activation(
                    out=stats[0:1, :], in_=ps[0:1, :], func=ACT.Copy, scale=1.0 / HW
                )

                # ---- transpose stats -> statsT [c(2x128), 2] ----
                pst = psM.tile([128, 2, 2], F32, tag="mlp")
                for j in range(2):
                    nc.tensor.transpose(
                        pst[:, j, :], stats[:, j * 128 : (j + 1) * 128], ident[0:2, 0:2]
                    )
                statsT = small.tile([128, 2, 2], F32, tag="statsT")
                nc.scalar.copy(out=statsT, in_=pst)

                # ---- MLP layer 1: h = relu(W1^T statsT + b1) ----
                ph = psM.tile([16, 2], F32, tag="mlp")
                for j in range(2):
                    nc.tensor.matmul(
                        ph,
                        lhsT=w1_sb[:, j, :],
                        rhs=statsT[:, j, :],
                        start=(j == 0),
                        stop=(j == 1),
                    )
                h_sb = small.tile([16, 2], F32, tag="h_sb")
                nc.scalar.activation(
                    out=h_sb, in_=ph, func=ACT.Relu, bias=b1_sb, scale=1.0
                )

                # ---- layer 2 + combine + sigmoid -> caT [256] in 2 chunks ----
                caT = small.tile([128, 2], F32, tag="caT")
                for j in range(2):
                    pc = psM.tile([128, 2], F32, tag="mlp")
                    nc.tensor.matmul(
                        pc, lhsT=w2_sb[:, j, :], rhs=h_sb, start=True, stop=True
                    )
                    pc_sb = small.tile([128, 2], F32, tag="pc_sb")
                    nc.scalar.copy(out=pc_sb, in_=pc)
                    catmp = small.tile([128, 1], F32, tag="catmp")
                    nc.vector.tensor_add(
                        out=catmp, in0=pc_sb[:, 0:1], in1=pc_sb[:, 1:2]
                    )
                    nc.scalar.activation(
                        out=caT[:, j : j + 1],
                        in_=catmp,
                        func=ACT.Sigmoid,
                        bias=b2x2[:, j : j + 1],
                        scale=1.0,
                    )

                # ---- broadcast ca over partitions: bca[:, b, :] ----
                pcr = psM.tile([1, 2, 128], F32, tag="mlp")
                for j in range(2):
                    nc.tensor.transpose(pcr[:, j, :], caT[:, j : j + 1], ident)
                ca_row = small.tile([1, 256], F32, tag="ca_row")
                nc.scalar.copy(out=ca_row, in_=pcr.rearrange("p j m -> p (j m)"))
                pbca = psB.tile([P, C], F32, tag="pbca", bufs=1)
                nc.tensor.matmul(pbca, lhsT=ones_r, rhs=ca_row, start=True, stop=True)
                nc.scalar.copy(out=bca[:, b, :], in_=pbca)

            # ---------------- phase B per image ----------------
            for b in (range(NIMG) if _stage() >= 3 else ()):
                maxc = mappool.tile([P, NB], F32, tag="maxc")
                sumc = mappool.tile([P, NB], F32, tag="sumc")
                # block 24 only covers partitions [0, 64); zero-fill the rest
                # so the map DMAs / scale op never touch uninitialized bytes
                nc.vector.memset(maxc, 0.0)
                nc.vector.memset(sumc, 0.0)

                use_sumc = int(os.environ.get("CBAM_SUMC", "1"))
                for t in range(NB):
                    pp = _pp(t)
                    nc.vector.tensor_mul(
                        out=X[0:pp, b, t, :],
                        in0=X[0:pp, b, t, :],
                        in1=bca[0:pp, b, :],
                    )
                    if use_sumc:
                        nc.scalar.activation(
                            out=scr[0:pp, :],
                            in_=X[0:pp, b, t, :],
                            func=ACT.Copy,
                            accum_out=sumc[0:pp, t : t + 1],
                        )
                # max over c: one 3D-AP reduce for the 24 full blocks, one for
                # the half block (innermost axis = c)
                nc.vector.tensor_reduce(
                    out=maxc[:, 0:NBF],
                    in_=X[:, b, 0:NBF, :],
                    axis=AX.X,
                    op=OP.max,
                )
                nc.vector.tensor_reduce(
                    out=maxc[0:HALF, NBF : NBF + 1],
                    in_=X[0:HALF, b, NBF : NBF + 1, :],
                    axis=AX.X,
                    op=OP.max,
                )
                # mean = sum / C
                nc.scalar.activation(
                    out=sumc, in_=sumc, func=ACT.Copy, scale=1.0 / C
                )

                # ---- rearrange maps: flat [128, 25] -> [56(w), 56(h)] ----
                mdr = dpool.tile([2, 3200], F32, tag="mdr")
                nc.sync.dma_start(
                    out=mdr[0, :].rearrange("(t p) -> p t", p=128), in_=sumc
                )
                nc.sync.dma_start(
                    out=mdr[1, :].rearrange("(t p) -> p t", p=128), in_=maxc
                )
                cin = work.tile([56, 2, 56], F32, tag="cin")
                for ch in range(2):
                    nc.sync.dma_start(
                        out=cin[:, ch, :],
                        in_=mdr[ch, 0:HW].rearrange("(h w) -> w h", w=56),
                    )

                if _bsub() < 3:
                    continue
                # ---- conv: 14 accumulated matmuls ----
                pconv = psB.tile([56, 56], F32, tag="pconv")
                dh_orders = ([0, -3, -2, -1, 1, 2, 3], [-3, -2, -1, 0, 1, 2, 3])
                first = True
                for ch in range(2):
                    for dh in dh_orders[ch]:
                        ho0 = max(0, -dh)
                        ho1 = 56 - max(0, dh)
                        last = ch == 1 and dh == 3
                        nc.tensor.matmul(
                            pconv[:, ho0:ho1],
                            lhsT=t_sb[:, ch * 7 + dh + 3, :],
                            rhs=cin[:, ch, ho0 + dh : ho1 + dh],
                            start=first,
                            stop=last,
                        )
                        first = False

                sawh = work.tile([56, 56], F32, tag="sawh")
                nc.scalar.activation(out=sawh, in_=pconv, func=ACT.Sigmoid)

                # ---- rearrange sa back to flat [128, 25] ----
                sdr = dpool.tile([3200], F32, tag="sdr")
                nc.sync.dma_start(
                    out=sdr[0:HW].rearrange("(h w) -> w h", w=56), in_=sawh
                )
                saf = mappool.tile([P, NB], F32, tag="saf")
                nc.sync.dma_start(
                    out=saf[:, 0:NBF],
                    in_=sdr[0 : NBF * P].rearrange("(t p) -> p t", p=128),
                )
                nc.sync.dma_start(
                    out=saf[0:HALF, NBF : NBF + 1],
                    in_=sdr[NBF * P : HW].rearrange("(p o) -> p o", o=1),
                )

                # ---- apply sa + DMA out ----
                if _bsub() < 4:
                    continue
                dve_apply = int(os.environ.get("CBAM_DVE_APPLY", "0"))
                for t in range(NB):
                    pp = _pp(t)
                    if dve_apply and t % 4 == 0:
                        nc.vector.tensor_scalar_mul(
                            out=X[0:pp, b, t, :],
                            in0=X[0:pp, b, t, :],
                            scalar1=saf[0:pp, t : t + 1],
                        )
                    else:
                        nc.scalar.activation(
                            out=X[0:pp, b, t, :],
                            in_=X[0:pp, b, t, :],
                            func=ACT.Copy,
                            scale=saf[0:pp, t : t + 1],
                        )
                nc.sync.dma_start(
                    out=out_hwc[b, 0 : NBF * P, :].rearrange("(t p) c -> p t c", p=128),
                    in_=X[:, b, 0:NBF, :],
                )
                nc.sync.dma_start(
                    out=out_hwc[b, NBF * P : HW, :], in_=X[0:HALF, b, NBF, :]
                )

            if _stage() < 3 or _bsub() < 4:
                # bisection passthrough: out = x (or xr for truncated phase B)
                for b in range(NIMG):
                    nc.sync.dma_start(
                        out=out_hwc[b, 0 : NBF * P, :].rearrange(
                            "(t p) c -> p t c", p=128
                        ),
                        in_=X[:, b, 0:NBF, :],
                    )
                    nc.sync.dma_start(
                        out=out_hwc[b, NBF * P : HW, :], in_=X[0:HALF, b, NBF, :]
                    )

    nc.finalize()
    return nc


LAST_RESULTS = None


def kernel(x, w1, b1, w2, b2, conv_w):
    global LAST_RESULTS
    nc = _CACHE.get("nc")
    if nc is None:
        nc = _build_nc()
        _CACHE["nc"] = nc

    x = np.ascontiguousarray(np.asarray(x, dtype=np.float32))
    shards = np.split(x, NCORES, axis=0)
    common = {
        "w1": np.ascontiguousarray(np.asarray(w1, dtype=np.float32)),
        "b1": np.ascontiguousarray(np.asarray(b1, dtype=np.float32)),
        "w2": np.ascontiguousarray(np.asarray(w2, dtype=np.float32)),
        "b2": np.ascontiguousarray(np.asarray(b2, dtype=np.float32)),
        "conv_w": np.ascontiguousarray(np.asarray(conv_w, dtype=np.float32)),
    }
    in_maps = [dict(common, x=np.ascontiguousarray(s)) for s in shards]

    res = run_bass_kernel_spmd(
        nc,
        in_maps,
        core_ids=list(range(NCORES)),
        trace=bool(int(os.environ.get("CBAM_TRACE", "0"))),
    )
    LAST_RESULTS = res
    return np.concatenate([r["out"] for r in res.results], axis=0)



# revision 18
# speedup vs baseline: 1.8489x; 1.8489x over previous
"""CBAM block (channel + spatial attention) Trainium2 Bass kernel.

Problem: x [32, 56, 56, 256] f32; data-parallel over batch across 8 NeuronCores
(4 images per core).  Everything is hardcoded for these shapes.

Per-core dataflow: 4 images = 2 image-PAIRS.  A pair is 2*3136 = 6272 = 128*49
rows, stored as X[p, q, t, c]: partition p in [0,128), pair q in [0,2),
t in [0,49), flat row within pair n = 49*p + t (so image 0 of the pair lives on
partitions [0,64), image 1 on [64,128)).  Each partition line of a pair DMA is
49*256*4 = 50KB contiguous DRAM -> near-peak HBM streaming.

  Phase A (channel stats, per pair):
    - max over hw: 6 DVE tensor_max folds into acc[128,7,256], strided reduce
      -> amax[128,256], PE transpose -> DVE 64-half reduce -> per-image maxes.
    - sum over hw: PE matmuls accumulated over t with one-hot-half lhsT
      (row 0 = image 0 sum, row 1 = image 1 sum).
  MLP per image (tiny): statsT [256c, (avg,max)] -> W1/relu/W2 -> sigmoid ->
    caT [256] -> PE transpose + per-half ones matmul -> combined bca [128,256].
  Phase B (per pair):
    - GPSIMD tensor_mul: X *= bca (in1 broadcast over t via stride-0 AP).
    - DVE tensor_reduce (axis=X): maxc [128,49] and sumc [128,49] per pixel.
  Spatial conv 7x7 (per image): maps -> DRAM -> [56(w), 56(h)] tiles;
    conv = 14 accumulated PE matmuls with per-(ch,dh) Toeplitz lhsT built from
    conv_w via 7 strided DMA band-writes into DRAM.  Sigmoid on ACT.
  Apply (per pair): X *= sa (in1 = saf[128,49] broadcast over c via stride-0
    AP); one DMA out per pair.
"""

import os

import numpy as np

import concourse.bass as bass
import concourse.bacc as bacc
import concourse.tile as tile
from concourse import mybir
from concourse.bass_utils import run_bass_kernel_spmd

F32 = mybir.dt.float32
AX = mybir.AxisListType
OP = mybir.AluOpType
ACT = mybir.ActivationFunctionType

P = 128          # partitions
NPAIR = 2        # image pairs per core
T = 49           # rows per partition per pair (6272 = 128*49)
HP = 64          # partitions per image within a pair
C = 256          # channels
HW = 3136        # 56*56
NCORES = 8

_CACHE: dict = {}


def _bcast_t(ap: bass.AP, nt: int) -> bass.AP:
    """[p, c] AP -> [p, nt, c] with stride-0 broadcast over the middle axis."""
    assert len(ap.ap) == 2
    return bass.AP(tensor=ap.tensor, offset=ap.offset,
                   ap=[ap.ap[0], [0, nt], ap.ap[1]])


def _bcast_c(ap: bass.AP, nc_: int) -> bass.AP:
    """[p, t] AP -> [p, t, nc] with stride-0 broadcast over the last axis."""
    assert len(ap.ap) == 2
    return bass.AP(tensor=ap.tensor, offset=ap.offset,
                   ap=[ap.ap[0], ap.ap[1], [0, nc_]])


def _build_nc() -> bass.Bass:
    nc = bacc.Bacc()

    x_d = nc.dram_tensor("x", [4, 56, 56, C], F32, kind="ExternalInput")
    w1_d = nc.dram_tensor("w1", [C, 16], F32, kind="ExternalInput")
    b1_d = nc.dram_tensor("b1", [16], F32, kind="ExternalInput")
    w2_d = nc.dram_tensor("w2", [16, C], F32, kind="ExternalInput")
    b2_d = nc.dram_tensor("b2", [C], F32, kind="ExternalInput")
    cw_d = nc.dram_tensor("conv_w", [7, 7, 2, 1], F32, kind="ExternalInput")
    out_d = nc.dram_tensor("out", [4, 56, 56, C], F32, kind="ExternalOutput")

    ident_d = nc.inline_tensor(np.eye(128, dtype=np.float32), name="ident128")
    # half-selectors: column/row i is 1 exactly on partitions of image i
    ohp_np = np.zeros((128, 2), dtype=np.float32)
    ohp_np[:HP, 0] = 1.0
    ohp_np[HP:, 1] = 1.0
    ohp_d = nc.inline_tensor(ohp_np, name="ohp")
    # sel_j[k=(i*2+jj), p] = 1 iff jj == j and p in image-i half; used to
    # broadcast ca4 [4(i,j), 128(c')] rows onto the right partitions/columns
    sel_np = np.zeros((2, 4, 128), dtype=np.float32)
    for j in range(2):
        for i in range(2):
            sel_np[j, i * 2 + j, i * HP : (i + 1) * HP] = 1.0
    sel0_d = nc.inline_tensor(sel_np[0], name="sel0")
    sel1_d = nc.inline_tensor(sel_np[1], name="sel1")

    # flat row-major views; pair q covers rows [6272q, 6272(q+1))
    x_flat = x_d[:].rearrange("b h w c -> (b h w) c")
    out_flat = out_d[:].rearrange("b h w c -> (b h w) c")

    with tile.TileContext(nc) as tc:
        import contextlib

        with contextlib.ExitStack() as ctx:
            cpool = ctx.enter_context(tc.tile_pool(name="cpool", bufs=1))
            xpool = ctx.enter_context(tc.tile_pool(name="xpool", bufs=1))
            work = ctx.enter_context(tc.tile_pool(name="work", bufs=2))
            small = ctx.enter_context(tc.tile_pool(name="small", bufs=4))
            mappool = ctx.enter_context(tc.tile_pool(name="mappool", bufs=2))
            psA = ctx.enter_context(tc.tile_pool(name="psA", bufs=2, space="PSUM"))
            psB = ctx.enter_context(tc.tile_pool(name="psB", bufs=2, space="PSUM"))
            psM = ctx.enter_context(tc.tile_pool(name="psM", bufs=2, space="PSUM"))
            dpool = ctx.enter_context(tc.tile_pool(name="dpool", bufs=2, space="DRAM"))
            dpool1 = ctx.enter_context(tc.tile_pool(name="dpool1", bufs=1, space="DRAM"))

            # ---------------- constants & weights ----------------
            ident = cpool.tile([128, 128], F32)
            nc.sync.dma_start(out=ident, in_=ident_d[:])

            w1_sb = cpool.tile([128, 2, 16], F32)
            nc.sync.dma_start(out=w1_sb, in_=w1_d[:].rearrange("(j p) m -> p j m", p=128))
            w2_sb = cpool.tile([16, 2, 128], F32)
            nc.sync.dma_start(out=w2_sb, in_=w2_d[:].rearrange("k (j m) -> k j m", j=2))
            b1_sb = cpool.tile([16, 1], F32)
            nc.sync.dma_start(out=b1_sb, in_=b1_d[:].rearrange("(p o) -> p o", o=1))
            b2_sb = cpool.tile([128, 2], F32)
            nc.sync.dma_start(out=b2_sb, in_=b2_d[:].rearrange("(j p) -> p j", p=128))
            b2x2 = cpool.tile([128, 2], F32)
            nc.scalar.activation(out=b2x2, in_=b2_sb, func=ACT.Copy, scale=2.0)

            # one-hot halves: col 0 selects image 0 (p<64), col 1 image 1
            ohp = cpool.tile([128, 2], F32)
            nc.sync.dma_start(out=ohp, in_=ohp_d[:])
            sel0 = cpool.tile([4, 128], F32)
            nc.sync.dma_start(out=sel0, in_=sel0_d[:])
            sel1 = cpool.tile([4, 128], F32)
            nc.sync.dma_start(out=sel1, in_=sel1_d[:])
            ones_r = cpool.tile([1, 128], F32)
            nc.vector.memset(ones_r, 1.0)

            # ---------------- Toeplitz conv matrices in DRAM ----------------
            # T[ch, dh][w_in, w_out] = conv_w[dh+3, dw+3, ch] where dw = w_in - w_out
            t_dram = dpool1.tile([2 * 7 * 56 * 56], F32)
            zt = cpool.tile([128, 343], F32)
            nc.vector.memset(zt, 0.0)
            nc.sync.dma_start(out=t_dram.rearrange("(p t) -> p t", p=128), in_=zt)
            # one band DMA per (ch, dw), dh folded into the APs
            for ch in range(2):
                for dwi in range(7):
                    dw = dwi - 3
                    cnt = 56 - abs(dw)
                    w0 = max(0, -dw)
                    dst = bass.AP(
                        tensor=t_dram.tensor,
                        offset=t_dram.offset + ch * 7 * HW + 57 * w0 + 56 * dw,
                        ap=[[HW, 7], [57, cnt]],
                    )
                    src = bass.AP(
                        tensor=cw_d,
                        offset=dwi * 2 + ch,
                        ap=[[14, 7], [0, cnt]],
                    )
                    nc.sync.dma_start(out=dst, in_=src)
            t_sb = cpool.tile([56, 14, 56], F32)
            nc.sync.dma_start(
                out=t_sb, in_=t_dram.rearrange("(m w v) -> w m v", m=14, w=56)
            )

            # ---------------- big SBUF state ----------------
            X = xpool.tile([P, NPAIR, T, C], F32)

            # PE warm-up matmuls touching every constant lhsT source so that
            # later matmuls don't accumulate one sync-wait per constant tensor
            # (the LDW struct has very few wait slots).
            pwu = psM.tile([128, 4], F32, tag="mlp")
            nc.tensor.matmul(pwu[0:2, 0:2], lhsT=ohp, rhs=ohp, start=True, stop=True)
            nc.tensor.matmul(
                pwu[0:128, 0:2], lhsT=sel0, rhs=sel0[:, 0:2], start=True, stop=True
            )
            nc.tensor.matmul(
                pwu[0:128, 0:2], lhsT=sel1, rhs=sel1[:, 0:2], start=True, stop=True
            )
            nc.tensor.matmul(
                pwu[0:4, 0:4], lhsT=ident[:, 0:4], rhs=ident[:, 0:4],
                start=True, stop=True,
            )
            nc.tensor.matmul(
                pwu[0:128, 0:1], lhsT=ones_r.rearrange("p m -> p m"),
                rhs=ones_r[:, 0:1], start=True, stop=True,
            )
            nc.tensor.matmul(
                pwu[0:4, 0:4], lhsT=t_sb[:, 0, 0:4], rhs=t_sb[:, 0, 0:4],
                start=True, stop=True,
            )
            nc.tensor.matmul(
                pwu[0:4, 0:4], lhsT=w1_sb[:, 0, 0:4], rhs=w1_sb[:, 0, 0:4],
                start=True, stop=True,
            )
            nc.tensor.matmul(
                pwu[0:4, 0:4], lhsT=w2_sb[:, 0, 0:4], rhs=w2_sb[:, 0, 0:4],
                start=True, stop=True,
            )

            # ---------------- DMA in (one per pair) ----------------
            for q in range(NPAIR):
                nc.sync.dma_start(
                    out=X[:, q, :, :],
                    in_=x_flat[q * P * T : (q + 1) * P * T, :].rearrange(
                        "(p t) c -> p t c", p=128
                    ),
                )

            bca_all = []  # per-pair combined channel-attention broadcast tiles
            for q in range(NPAIR):
                # ---------------- phase A: stats ----------------
                # max over hw: fold 49 t-slices into 7, then strided reduce
                acc = work.tile([P, 7, C], F32, tag="acc")
                nc.vector.tensor_max(
                    out=acc, in0=X[:, q, 0:7, :], in1=X[:, q, 7:14, :]
                )
                for k in range(2, 7):
                    nc.vector.tensor_max(
                        out=acc, in0=acc, in1=X[:, q, 7 * k : 7 * (k + 1), :]
                    )
                amax = work.tile([P, C], F32, tag="amax")
                nc.vector.tensor_reduce(
                    out=amax, in_=acc.rearrange("p t c -> p c t"),
                    axis=AX.X, op=OP.max,
                )
                # partition max per image half: transpose then 64-wide reduces
                pamx = psM.tile([128, 2, 128], F32, tag="mlp")
                for j in range(2):
                    nc.tensor.transpose(
                        pamx[:, j, :], amax[:, j * 128 : (j + 1) * 128], ident
                    )
                mxT = small.tile([128, 2, 2], F32, tag="mxT")
                nc.vector.tensor_reduce(
                    out=mxT,
                    in_=pamx.rearrange("p j (i h) -> p j i h", i=2),
                    axis=AX.X, op=OP.max,
                )

                # sum over hw on PE; psum row 0 = image 0, row 1 = image 1
                ps = psA.tile([2, C], F32, tag="ps_sum")
                for t in range(T):
                    nc.tensor.matmul(
                        ps, lhsT=ohp, rhs=X[:, q, t, :],
                        start=(t == 0), stop=(t == T - 1),
                    )
                ssum = small.tile([2, C], F32, tag="ssum")
                nc.scalar.activation(
                    out=ssum, in_=ps, func=ACT.Copy, scale=1.0 / HW
                )

                # transpose both avg rows at once: pavg[c128, j, i]
                pavg = psM.tile([128, 2, 2], F32, tag="mlp")
                for j in range(2):
                    nc.tensor.transpose(
                        pavg[:, j, :], ssum[:, j * 128 : (j + 1) * 128], ident[0:2, 0:2]
                    )
                # statsT per image [c(2x128), j, (avg, max)] — consume pavg now
                statsTs = []
                for i in range(2):
                    statsT = small.tile([128, 2, 2], F32, tag=f"statsT{i}")
                    nc.scalar.copy(out=statsT[:, :, 0:1], in_=pavg[:, :, i : i + 1])
                    nc.scalar.copy(out=statsT[:, :, 1:2], in_=mxT[:, :, i : i + 1])
                    statsTs.append(statsT)

                # ---------------- MLP per image ----------------
                # caT_both columns: (i*2+j) = image i, channel-chunk j
                caT_both = small.tile([128, 4], F32, tag="caT_both")
                for i in range(2):
                    statsT = statsTs[i]

                    # layer 1: h = relu(W1^T statsT + b1)   [16, 2]
                    ph = psM.tile([16, 2], F32, tag="mlp")
                    for j in range(2):
                        nc.tensor.matmul(
                            ph, lhsT=w1_sb[:, j, :], rhs=statsT[:, j, :],
                            start=(j == 0), stop=(j == 1),
                        )
                    h_sb = small.tile([16, 2], F32, tag="h_sb")
                    nc.scalar.activation(
                        out=h_sb, in_=ph, func=ACT.Relu, bias=b1_sb, scale=1.0
                    )

                    # layer 2 + combine + sigmoid -> caT_both cols (2i, 2i+1)
                    for j in range(2):
                        pc = psM.tile([128, 2], F32, tag="mlp")
                        nc.tensor.matmul(
                            pc, lhsT=w2_sb[:, j, :], rhs=h_sb, start=True, stop=True
                        )
                        pc_sb = small.tile([128, 2], F32, tag="pc_sb")
                        nc.scalar.copy(out=pc_sb, in_=pc)
                        catmp = small.tile([128, 1], F32, tag="catmp")
                        nc.vector.tensor_add(
                            out=catmp, in0=pc_sb[:, 0:1], in1=pc_sb[:, 1:2]
                        )
                        nc.scalar.activation(
                            out=caT_both[:, 2 * i + j : 2 * i + j + 1],
                            in_=catmp, func=ACT.Sigmoid,
                            bias=b2x2[:, j : j + 1], scale=1.0,
                        )

                # transpose once: ca4 [4(i,j), 128(c')]
                pca4 = psM.tile([4, 128], F32, tag="mlp")
                nc.tensor.transpose(pca4, caT_both, ident)
                ca4 = small.tile([4, 128], F32, tag="ca4")
                nc.scalar.copy(out=ca4, in_=pca4)

                # combined bca: partitions [0,64) get image0 ca, [64,128) image1
                pbca = psB.tile([P, 2, 128], F32, tag="pbca")
                nc.tensor.matmul(pbca[:, 0, :], lhsT=sel0, rhs=ca4, start=True, stop=True)
                nc.tensor.matmul(pbca[:, 1, :], lhsT=sel1, rhs=ca4, start=True, stop=True)
                bca = work.tile([P, C], F32, tag="bca")
                nc.scalar.copy(out=bca, in_=pbca.rearrange("p j m -> p (j m)"))
                bca_all.append(bca)

            for q in range(NPAIR):
                bca = bca_all[q]
                # ---------------- phase B: xr = x * ca, per-pixel stats ----
                nc.gpsimd.tensor_mul(
                    out=X[:, q, :, :],
                    in0=X[:, q, :, :],
                    in1=_bcast_t(bca[:, :], T),
                )
                maxc = mappool.tile([P, T], F32, tag="maxc")
                nc.vector.tensor_reduce(
                    out=maxc, in_=X[:, q, :, :], axis=AX.X, op=OP.max
                )
                sumc = mappool.tile([P, T], F32, tag="sumc")
                nc.vector.tensor_reduce(
                    out=sumc, in_=X[:, q, :, :], axis=AX.X, op=OP.add
                )
                nc.scalar.activation(
                    out=sumc, in_=sumc, func=ACT.Copy, scale=1.0 / C
                )

                # ---- rearrange maps: [128, 49] -> per-image [56(w), 56(h)] ----
                mdr = dpool.tile([2, P * T], F32, tag="mdr")
                nc.sync.dma_start(
                    out=mdr[0, :].rearrange("(p t) -> p t", p=128), in_=sumc
                )
                nc.sync.dma_start(
                    out=mdr[1, :].rearrange("(p t) -> p t", p=128), in_=maxc
                )
                sdr = dpool.tile([P * T], F32, tag="sdr")
                for i in range(2):
                    cin = work.tile([56, 2, 56], F32, tag="cin")
                    for ch in range(2):
                        nc.sync.dma_start(
                            out=cin[:, ch, :],
                            in_=mdr[ch, i * HW : (i + 1) * HW].rearrange(
                                "(h w) -> w h", w=56
                            ),
                        )
                    # ---- conv: 14 accumulated matmuls ----
                    pconv = psB.tile([56, 56], F32, tag="pconv")
                    dh_orders = ([0, -3, -2, -1, 1, 2, 3], [-3, -2, -1, 0, 1, 2, 3])
                    first = True
                    for ch in range(2):
                        for dh in dh_orders[ch]:
                            ho0 = max(0, -dh)
                            ho1 = 56 - max(0, dh)
                            last = ch == 1 and dh == 3
                            nc.tensor.matmul(
                                pconv[:, ho0:ho1],
                                lhsT=t_sb[:, ch * 7 + dh + 3, :],
                                rhs=cin[:, ch, ho0 + dh : ho1 + dh],
                                start=first, stop=last,
                            )
                            first = False
                    sawh = work.tile([56, 56], F32, tag="sawh")
                    nc.scalar.activation(out=sawh, in_=pconv, func=ACT.Sigmoid)
                    nc.sync.dma_start(
                        out=sdr[i * HW : (i + 1) * HW].rearrange("(h w) -> w h", w=56),
                        in_=sawh,
                    )

                saf = mappool.tile([P, T], F32, tag="saf")
                nc.sync.dma_start(
                    out=saf, in_=sdr.rearrange("(p t) -> p t", p=128)
                )

                # ---------------- apply + DMA out ----------------
                nc.vector.tensor_mul(
                    out=X[:, q, :, :],
                    in0=X[:, q, :, :],
                    in1=_bcast_c(saf[:, :], C),
                )
                nc.sync.dma_start(
                    out=out_flat[q * P * T : (q + 1) * P * T, :].rearrange(
                        "(p t) c -> p t c", p=128
                    ),
                    in_=X[:, q, :, :],
                )

    nc.finalize()
    return nc


LAST_RESULTS = None


def kernel(x, w1, b1, w2, b2, conv_w):
    global LAST_RESULTS
    nc = _CACHE.get("nc")
    if nc is None:
        nc = _build_nc()
        _CACHE["nc"] = nc

    x = np.ascontiguousarray(np.asarray(x, dtype=np.float32))
    shards = np.split(x, NCORES, axis=0)
    common = {
        "w1": np.ascontiguousarray(np.asarray(w1, dtype=np.float32)),
        "b1": np.ascontiguousarray(np.asarray(b1, dtype=np.float32)),
        "w2": np.ascontiguousarray(np.asarray(w2, dtype=np.float32)),
        "b2": np.ascontiguousarray(np.asarray(b2, dtype=np.float32)),
        "conv_w": np.ascontiguousarray(np.asarray(conv_w, dtype=np.float32)),
    }
    in_maps = [dict(common, x=np.ascontiguousarray(s)) for s in shards]

    res = run_bass_kernel_spmd(
        nc,
        in_maps,
        core_ids=list(range(NCORES)),
        trace=bool(int(os.environ.get("CBAM_TRACE", "0"))),
    )
    LAST_RESULTS = res
    return np.concatenate([r["out"] for r in res.results], axis=0)
